# revision 17
# baseline (speedup 1.0000x reference)
"""Trainium2 Bass kernel for a ViT-Base transformer encoder block.

Input x: [64, 197, 768] fp32 + weights. Data-parallel over batch across 8
NeuronCores (8 batches/core = 1576 tokens/core). All matmul operands are
bf16 (fp32 PSUM accumulation); weights and x are cast to bf16 host-side,
and x is additionally passed pre-transposed (d-major) so no PE transposes
are needed in pass 1.

Per core, two passes over 4 batch-pairs (2 batches = 394 tokens each):

  pass 1: QKV projections, software-pipelined attention (per-batch
          197-col matmuls; odd heads write PSUM partitions 64:128 via
          tile_position; softmax denominators via rowsum matmuls +
          reciprocal_approx_fast + gpsimd partition_broadcast),
          O-projection, LayerNorm1 + residual -> x1 kept in SBUF (bf16).
          Pair p+1's projections are interleaved into pair p's attention
          pipeline to keep the in-order PE queue dense.
  pass 2: MLP with W1/W2 resident in SBUF (bf16), exact GELU fused into
          the PSUM eviction, PE transpose back to token-major,
          LayerNorm2 + residual -> out.
"""
import os
import sys

sys.path.insert(0, "/opt/trn_rl_repo")

import numpy as np
import ml_dtypes
from contextlib import ExitStack

import concourse.bass as bass
import concourse.tile as tile
from concourse import bacc, mybir
from concourse.bass_utils import run_bass_kernel_spmd
from concourse.masks import make_identity

DIM, NH, HD, HID = 768, 12, 64, 3072
S = 197
B = 64
N_CORES = 8
BPC = B // N_CORES            # 8 batches per core
T = BPC * S                   # 1576 tokens per core
NPAIR = BPC // 2              # 4 batch pairs per core
PT = 2 * S                    # 394 tokens per pair
EPS = 1e-6
DC = DIM // 128               # 6 d-chunks
HC = HID // 128               # 24 hidden chunks

F32 = mybir.dt.float32
BF16 = mybir.dt.bfloat16
AF = mybir.ActivationFunctionType
OP = mybir.AluOpType

# token tiles within a pair: (offset, size); tile 2*b + s for batch b
Q_TILES = [(0, 128), (128, 69), (197, 128), (325, 69)]

DEBUG = bool(int(os.environ.get("BASSK_DEBUG", "0")))

_cached = None


def _build():
    nc = bacc.Bacc("TRN2", target_bir_lowering=False, debug=False)

    # host-side pre-shuffled layouts: weights [128, C*out], xt [128, DC*T],
    # broadcast biases pre-tiled [128, 7*DIM], per-partition biases [128, 36]
    x_d = nc.dram_tensor("x", [T, DIM], BF16, kind="ExternalInput").ap()
    xt_d = nc.dram_tensor("xt", [128, DC * T], BF16, kind="ExternalInput").ap()
    w_d = {}
    for name, shape, dt in [("Wq", [128, DC * DIM], BF16),
                            ("Wk", [128, DC * DIM], BF16),
                            ("Wv", [128, DC * DIM], BF16),
                            ("Wo", [128, DC * DIM], BF16),
                            ("W1", [128, DC * HID], BF16),
                            ("W2", [128, HC * DIM], BF16),
                            ("bcb", [128, 7 * DIM], BF16),
                            ("bpp", [128, 36], F32)]:
        w_d[name] = nc.dram_tensor(name, shape, dt, kind="ExternalInput").ap()
    out_d = nc.dram_tensor("out", [T, DIM], F32, kind="ExternalOutput").ap()

    dbg = {}
    if DEBUG:
        for name, shape in [("dq", [DIM, PT]), ("dk", [DIM, PT]),
                            ("dv", [PT, DIM]), ("dctx", [DIM, PT]),
                            ("dx1", [PT, DIM]), ("dh", [HID, PT]),
                            ("dex", [NH, 2, 128, PT])]:
            dbg[name] = nc.dram_tensor(name, shape, BF16, kind="ExternalOutput").ap()
        dbg["drt"] = nc.dram_tensor("drt", [NH, PT], F32, kind="ExternalOutput").ap()
        dbg["dpc"] = nc.dram_tensor("dpc", [DC, 128, PT], F32, kind="ExternalOutput").ap()

    with tile.TileContext(nc) as tc, ExitStack() as octx:
        persist = octx.enter_context(tc.tile_pool(name="persist", bufs=1))

        # ---- constants ----
        ident_f = persist.tile([128, 128], F32)
        make_identity(nc, ident_f[:])
        ident_b = persist.tile([128, 128], BF16)
        nc.vector.tensor_copy(ident_b[:], ident_f[:])
        ones_b = persist.tile([128, 1], BF16)
        nc.vector.memset(ones_b[:], 1.0)
        eps_sb = persist.tile([128, 1], F32)
        nc.vector.memset(eps_sb[:], EPS)

        # per-partition bias layouts, one packed DMA (bq | bk | b1)
        bpp = persist.tile([128, 36], F32)
        nc.sync.dma_start(bpp[:], w_d["bpp"])
        bq_sb = bpp[:, 0:DC]
        bk_sb = bpp[:, DC:2 * DC]
        b1_sb = bpp[:, 2 * DC:2 * DC + HC]

        # broadcast-[128, 768] biases (bf16), one packed DMA
        bcb = persist.tile([128, 7, DIM], BF16)
        nc.sync.dma_start(bcb[:], w_d["bcb"].rearrange("p (a j) -> p a j", j=DIM))
        bcast = {name: bcb[:, i, :]
                 for i, name in enumerate(
                     ["bv", "bo", "b2", "g1", "be1", "g2", "be2"])}

        # attention weights, resident bf16 (DMAs issued after the x loads,
        # on the scalar-engine queue so they overlap the x DMAs on sync)
        Wt = {}
        for name in ["Wq", "Wk", "Wv", "Wo"]:
            wt = persist.tile([128, DC, DIM], BF16, name=f"wt_{name}", tag=f"wt_{name}")
            Wt[name] = wt

        # W1 prefetched during pass 1 (needed at pass-2 start)
        W1t = persist.tile([128, DC, HID], BF16)

        # x1 = attn block output, kept in SBUF across passes (bf16)
        x1_all = persist.tile([128, NPAIR, 4, DIM], BF16)

        xt_view = xt_d.rearrange("p (c t) -> p c t", t=T)

        # =========================== PASS 1 ===========================
        with ExitStack() as ctx:
            xp = ctx.enter_context(tc.tile_pool(name="xp", bufs=3))
            big = ctx.enter_context(tc.tile_pool(name="p1big", bufs=2))
            exp_pool = ctx.enter_context(tc.tile_pool(name="exp", bufs=6))
            rt_pool = ctx.enter_context(tc.tile_pool(name="rt", bufs=2))
            bc_pool = ctx.enter_context(tc.tile_pool(name="bc", bufs=3))
            ao_pool = ctx.enter_context(tc.tile_pool(name="ao", bufs=2))
            ln_pool = ctx.enter_context(tc.tile_pool(name="ln", bufs=3))

            xfp = ctx.enter_context(tc.tile_pool(name="xfp", bufs=1))
            # full d-major x, loaded once (chunked DMAs for fast queue drain)
            xT_full = xfp.tile([128, DC, T], BF16)

            ps_mm = ctx.enter_context(tc.tile_pool(name="psmm", bufs=2, space="PSUM"))
            ps_sp = ctx.enter_context(tc.tile_pool(name="pssp", bufs=4, space="PSUM"))
            ps_cx = ctx.enter_context(tc.tile_pool(name="pscx", bufs=2, space="PSUM"))

            def load_x(p):
                g0 = p * PT
                x_sb = xp.tile([128, 4, DIM], BF16, tag="x", name="x_sb")
                for i, (off, sz) in enumerate(Q_TILES):
                    nc.sync.dma_start(x_sb[0:sz, i, :], x_d[g0 + off:g0 + off + sz, :])
                return x_sb

            # --- projection work items for one pair (list of thunks) ---
            def proj_items(p, xT, dst):
                g0 = p * PT
                qT = big.tile([128, DC, PT], BF16, tag="qT", name="qT")
                kT = big.tile([128, DC, PT], BF16, tag="kT", name="kT")
                v_sb = big.tile([128, 4, NH, HD], BF16, tag="v", name="v_sb")
                dst["qT"], dst["kT"], dst["v_sb"] = qT, kT, v_sb
                items = []

                def qk_item(wname, bsb, dstT, c):
                    def run():
                        pm = ps_mm.tile([128, 512], F32, tag="mm", name="pm")
                        for kc in range(DC):
                            nc.tensor.matmul(pm[:, 0:PT],
                                             Wt[wname][:, kc, c * 128:(c + 1) * 128],
                                             xT[:, kc, g0:g0 + PT],
                                             start=(kc == 0), stop=(kc == DC - 1))
                        with nc.allow_low_precision(reason="qk bf16"):
                            nc.vector.tensor_scalar(dstT[:, c, :], pm[:, 0:PT],
                                                    bsb[:, c:c + 1], None, OP.add)
                    return run

                def v_item(i, s):
                    def run():
                        off, sz = Q_TILES[i]
                        pm = ps_mm.tile([128, 512], F32, tag="mm", name="pm")
                        for kc in range(DC):
                            nc.tensor.matmul(pm[0:sz, 0:384],
                                             xT[:, kc, g0 + off:g0 + off + sz],
                                             Wt["Wv"][:, kc, s * 384:(s + 1) * 384],
                                             start=(kc == 0), stop=(kc == DC - 1))
                        with nc.allow_low_precision(reason="v bf16"):
                            nc.vector.tensor_add(
                                v_sb[0:sz, i, 6 * s:6 * s + 6, :],
                                pm[0:sz, 0:384].rearrange("p (a b) -> p a b", a=6),
                                bcast["bv"][0:sz, s * 384:(s + 1) * 384]
                                    .rearrange("p (a b) -> p a b", a=6))
                    return run

                for c in range(DC):
                    items.append(qk_item("Wq", bq_sb, qT, c))
                    items.append(qk_item("Wk", bk_sb, kT, c))
                for i in range(4):
                    for s in range(2):
                        items.append(v_item(i, s))
                return items

            # --- attention for pair p, interleaving `items` (next pair's
            #     projections) between pipeline steps ---
            def attention(p, cur, items):
                qT, kT, v_sb = cur["qT"], cur["kT"], cur["v_sb"]
                ctxT = big.tile([128, DC, PT], BF16, tag="ctxT", name="ctxT")
                cur["ctxT"] = ctxT
                it = 0
                NSTEP = NH + 2
                exps = {}   # h -> [exp_s0, exp_s1]
                pcs = {}    # hc -> psum tile
                bcs = {}    # h -> bc tile
                for step in range(NSTEP):
                    # stage S: scores + exp for head `step`
                    if step < NH:
                        h = step
                        hc, hp = h // 2, (h % 2) * 64
                        exps[h] = []
                        for s in range(2):
                            ksz = Q_TILES[s][1]
                            psc = ps_sp.tile([128, 512], F32, tag="sp", name="psc")
                            for b in range(2):
                                koff = Q_TILES[2 * b + s][0]
                                cs = slice(b * S, (b + 1) * S)
                                nc.tensor.matmul(
                                    psc[0:ksz, cs],
                                    kT[hp:hp + 64, hc, koff:koff + ksz],
                                    qT[hp:hp + 64, hc, cs],
                                    start=True, stop=True,
                                    skip_group_check=True)
                            et = exp_pool.tile([128, 394], BF16, tag="exp", name="et")
                            with nc.allow_low_precision(reason="softmax exp bf16"):
                                nc.scalar.activation(et[0:ksz, :], psc[0:ksz, 0:PT],
                                                     AF.Exp, bias=0.0, scale=0.125)
                            exps[h].append(et)
                            if DEBUG and p == 0:
                                nc.sync.dma_start(dbg["dex"][h, s, :, :], et[:, :])
                    # stage R: rowsum + recip + broadcast for head step-1
                    if 1 <= step <= NH:
                        h = step - 1
                        hc, hp = h // 2, (h % 2) * 64
                        pq = ps_sp.tile([128, 512], F32, tag="sp", name="pq")
                        for s in range(2):
                            ksz = Q_TILES[s][1]
                            nc.tensor.matmul(
                                pq[0:1, 0:PT],
                                ones_b[0:ksz, 0:1],
                                exps[h][s][0:ksz, :],
                                start=(s == 0), stop=(s == 1),
                                skip_group_check=True)
                        rt = rt_pool.tile([128, 394], F32, tag="rt", name="rt")
                        with nc.allow_low_precision(reason="softmax recip"):
                            nc.vector.reciprocal_approx_fast(
                                rt[0:1, :], pq[0:1, 0:PT])
                        if DEBUG and p == 0:
                            nc.sync.dma_start(dbg["drt"][h, :], rt[0:1, :])
                        bc = bc_pool.tile([128, 394], F32, tag="bcsb", name="bc")
                        nc.gpsimd.partition_broadcast(bc[:, :], rt[0:1, :],
                                                      channels=128)
                        bcs[h] = bc
                    # stage C: ctx for head step-1
                    if 1 <= step <= NH:
                        h = step - 1
                        hc, hp = h // 2, (h % 2) * 64
                        if hp == 0:
                            pcs[hc] = ps_cx.tile([128, 512], F32, tag="cx",
                                                 name=f"cx{hc}")
                        pc = pcs[hc]
                        for b in range(2):
                            cs = slice(b * S, (b + 1) * S)
                            for s in range(2):
                                ksz = Q_TILES[s][1]
                                nc.tensor.matmul(
                                    pc[hp:hp + 64, cs],
                                    v_sb[0:ksz, 2 * b + s, h, :],
                                    exps[h][s][0:ksz, cs],
                                    start=(s == 0), stop=(s == 1),
                                    skip_group_check=True)
                    # interleave next-pair projection work
                    budget = 2 if step < NSTEP - 1 else len(items) - it
                    for _ in range(budget):
                        if it < len(items):
                            items[it]()
                            it += 1
                    # stage N: normalize head step-2
                    if step >= 2:
                        h = step - 2
                        hc, hp = h // 2, (h % 2) * 64
                        if DEBUG and p == 0 and hp == 64:
                            dt_ = bc_pool.tile([128, 394], F32, tag="bcsb",
                                               name="dt_")
                            nc.scalar.activation(dt_[:, :], pcs[hc][:, 0:PT],
                                                 AF.Copy, bias=0.0, scale=1.0)
                            nc.sync.dma_start(dbg["dpc"][hc, :, :], dt_[:, :])
                        with nc.allow_low_precision(reason="ctx bf16"):
                            nc.vector.tensor_tensor(
                                ctxT[hp:hp + 64, hc, :],
                                pcs[hc][hp:hp + 64, 0:PT],
                                bcs[h][hp:hp + 64, :], OP.mult)
                while it < len(items):
                    items[it]()
                    it += 1

            def o_proj_ln1(p, cur):
                ctxT, x_sb = cur["ctxT"], cur["x_sb"]
                for i, (off, sz) in enumerate(Q_TILES):
                    ao = ao_pool.tile([128, DIM], F32, tag="ao", name="ao")
                    for s in range(2):
                        pm = ps_mm.tile([128, 512], F32, tag="mm", name="pm")
                        for c in range(DC):
                            nc.tensor.matmul(pm[0:sz, 0:384],
                                             ctxT[:, c, off:off + sz],
                                             Wt["Wo"][:, c, s * 384:(s + 1) * 384],
                                             start=(c == 0), stop=(c == DC - 1))
                        nc.vector.tensor_add(ao[0:sz, s * 384:(s + 1) * 384],
                                             pm[0:sz, 0:384],
                                             bcast["bo"][0:sz, s * 384:(s + 1) * 384])
                    # LayerNorm 1
                    st = ln_pool.tile([128, 3, nc.vector.BN_STATS_DIM], F32, tag="st",
                                      name="st")
                    for g in range(3):
                        nc.vector.bn_stats(st[0:sz, g, :], ao[0:sz, g * 256:(g + 1) * 256])
                    mv = ln_pool.tile([128, nc.vector.BN_AGGR_DIM], F32, tag="mv",
                                      name="mv")
                    nc.vector.bn_aggr(mv[0:sz, :], st[0:sz, :, :])
                    sd = ln_pool.tile([128, 2], F32, tag="sd", name="sd")
                    nc.scalar.activation(sd[0:sz, 0:1], mv[0:sz, 1:2], AF.Sqrt,
                                         bias=eps_sb[0:sz, :], scale=1.0)
                    rstd = ln_pool.tile([128, 1], F32, tag="rstd", name="rstd")
                    with nc.allow_low_precision(reason="ln1 recip"):
                        nc.vector.reciprocal_approx_fast(rstd[0:sz, :], sd[0:sz, 0:1])
                    nmr = ln_pool.tile([128, 1], F32, tag="nmr", name="nmr")
                    nc.vector.tensor_scalar(nmr[0:sz, :], mv[0:sz, 0:1],
                                            rstd[0:sz, :], -1.0, OP.mult, OP.mult)
                    nc.scalar.activation(ao[0:sz, :], ao[0:sz, :], AF.Identity,
                                         bias=nmr[0:sz, :], scale=rstd[0:sz, :])
                    nc.vector.tensor_tensor(ao[0:sz, :], ao[0:sz, :],
                                            bcast["g1"][0:sz, :], OP.mult)
                    nc.vector.tensor_add(ao[0:sz, :], ao[0:sz, :], x_sb[0:sz, i, :])
                    with nc.allow_low_precision(reason="x1 bf16"):
                        nc.vector.tensor_add(x1_all[0:sz, p, i, :], ao[0:sz, :],
                                             bcast["be1"][0:sz, :])

            # ---- pass-1 driver: pipelined over pairs ----
            for c_ in range(DC):
                nc.sync.dma_start(xT_full[:, c_, :], xt_view[:, c_, :])
            for name in ["Wq", "Wk", "Wv", "Wo"]:
                nc.scalar.dma_start(
                    Wt[name][:], w_d[name].rearrange("p (c j) -> p c j", j=DIM))
            nc.gpsimd.dma_start(
                W1t[:], w_d["W1"].rearrange("p (c j) -> p c j", j=HID))
            cur = {}
            x_sb0 = load_x(0)
            nxt_x = load_x(1)
            cur["x_sb"] = x_sb0
            for item in proj_items(0, xT_full, cur):
                item()
            for p in range(NPAIR):
                if p + 1 < NPAIR:
                    nxt = {"x_sb": nxt_x}
                    items = proj_items(p + 1, xT_full, nxt)
                else:
                    nxt = None
                    items = []
                attention(p, cur, items)
                if p + 2 < NPAIR:
                    nxt_x = load_x(p + 2)
                o_proj_ln1(p, cur)

                if DEBUG and p == 0:
                    for c in range(DC):
                        nc.sync.dma_start(dbg["dq"][c * 128:(c + 1) * 128, :],
                                          cur["qT"][:, c, :])
                        nc.sync.dma_start(dbg["dk"][c * 128:(c + 1) * 128, :],
                                          cur["kT"][:, c, :])
                        nc.sync.dma_start(dbg["dctx"][c * 128:(c + 1) * 128, :],
                                          cur["ctxT"][:, c, :])
                    for i, (off, sz) in enumerate(Q_TILES):
                        nc.sync.dma_start(dbg["dv"][off:off + sz, :],
                                          cur["v_sb"][0:sz, i, :, :])
                        nc.sync.dma_start(dbg["dx1"][off:off + sz, :],
                                          x1_all[0:sz, 0, i, :])
                cur = nxt

        # =========================== PASS 2 ===========================
        with ExitStack() as ctx:
            wpool = ctx.enter_context(tc.tile_pool(name="w2p", bufs=1))
            W2t = wpool.tile([128, HC, DIM], BF16)
            nc.gpsimd.dma_start(W2t[:], w_d["W2"].rearrange("p (c j) -> p c j", j=DIM))

            xtp = ctx.enter_context(tc.tile_pool(name="xtp", bufs=2))
            htp = ctx.enter_context(tc.tile_pool(name="htp", bufs=1))
            mo_pool = ctx.enter_context(tc.tile_pool(name="mo", bufs=2))
            moT_pool = ctx.enter_context(tc.tile_pool(name="moT", bufs=2))
            ln_pool = ctx.enter_context(tc.tile_pool(name="ln2", bufs=3))
            out_pool = ctx.enter_context(tc.tile_pool(name="outp", bufs=2))

            ps_wk = ctx.enter_context(tc.tile_pool(name="pswk", bufs=2, space="PSUM"))
            ps_ac = ctx.enter_context(tc.tile_pool(name="psac", bufs=6, space="PSUM"))

            def x1t_items(p, box):
                x1T = xtp.tile([128, DC, PT], BF16, tag="x1T", name="x1T")
                box["x1T"] = x1T

                def one(i):
                    def run():
                        off, sz = Q_TILES[i]
                        for c in range(DC):
                            pt = ps_wk.tile([128, 512], F32, tag="wk", name="pt")
                            ptb = pt[:, 0:64].bitcast(BF16)
                            nc.tensor.transpose(
                                ptb[:, 0:sz],
                                x1_all[0:sz, p, i, c * 128:(c + 1) * 128],
                                ident_b[0:sz, 0:sz])
                            nc.vector.tensor_copy(x1T[:, c, off:off + sz],
                                                  ptb[:, 0:sz])
                    return run
                return [one(i) for i in range(4)]

            LAG = 6

            def mlp(p, x1T, evict_items, tail_its):
                hT = htp.tile([128, HC, PT], BF16, tag="hT", name="hT")
                pacs = [ps_ac.tile([128, 512], F32, tag="ac", name=f"pac{c}")
                        for c in range(DC)]
                ti = 0
                for hcx in range(HC + LAG):
                    if hcx < HC:
                        pm = ps_wk.tile([128, 512], F32, tag="wk", name="pm")
                        for kc in range(DC):
                            nc.tensor.matmul(pm[:, 0:PT],
                                             W1t[:, kc, hcx * 128:(hcx + 1) * 128],
                                             x1T[:, kc, :],
                                             start=(kc == 0), stop=(kc == DC - 1))
                        with nc.allow_low_precision(reason="h bf16"):
                            nc.scalar.activation(hT[:, hcx, :], pm[:, 0:PT], AF.Gelu,
                                                 bias=b1_sb[:, hcx:hcx + 1], scale=1.0)
                        if hcx < len(evict_items):
                            evict_items[hcx]()
                    h2 = hcx - LAG
                    if h2 >= 0:
                        for c in range(DC):
                            nc.tensor.matmul(pacs[c][:, 0:PT],
                                             W2t[:, h2, c * 128:(c + 1) * 128],
                                             hT[:, h2, :],
                                             start=(h2 == 0), stop=(h2 == HC - 1))
                    if hcx >= LAG and hcx % 2 == 0 and ti < len(tail_its):
                        tail_its[ti]()
                        ti += 1
                while ti < len(tail_its):
                    tail_its[ti]()
                    ti += 1
                if DEBUG and p == 0:
                    for hcx in range(HC):
                        nc.sync.dma_start(dbg["dh"][hcx * 128:(hcx + 1) * 128, :],
                                          hT[:, hcx, :])
                return pacs

            def evict_items_for(p, pacs, box):
                moT = moT_pool.tile([128, DC, PT], BF16, tag="moT", name="moT")
                box["moT"] = moT

                def one(c):
                    def run():
                        with nc.allow_low_precision(reason="moT bf16"):
                            nc.scalar.activation(moT[:, c, :], pacs[c][:, 0:PT],
                                                 AF.Copy, bias=0.0, scale=1.0)
                    return run
                return [one(c) for c in range(DC)]

            def tail_items_for(p, box):
                g0 = p * PT
                moT = box["moT"]

                def one(i):
                    def run():
                        off, sz = Q_TILES[i]
                        _ln2_tile(p, g0, moT, i, off, sz)
                    return run
                return [one(i) for i in range(4)]

            def _ln2_tile(p, g0, moT, i, off, sz):
                if True:
                    mo = mo_pool.tile([128, DIM], F32, tag="mo", name="mo")
                    for c in range(DC):
                        pt = ps_wk.tile([128, 512], F32, tag="wk", name="pt")
                        ptb = pt[:, 0:64].bitcast(BF16)
                        nc.tensor.transpose(ptb[0:sz, 0:128],
                                            moT[:, c, off:off + sz], ident_b[:, :])
                        nc.vector.tensor_copy(mo[0:sz, c * 128:(c + 1) * 128],
                                              ptb[0:sz, 0:128])
                    nc.vector.tensor_add(mo[0:sz, :], mo[0:sz, :], bcast["b2"][0:sz, :])
                    # LayerNorm 2 + residual
                    st = ln_pool.tile([128, 3, nc.vector.BN_STATS_DIM], F32, tag="st",
                                      name="st")
                    for g in range(3):
                        nc.vector.bn_stats(st[0:sz, g, :], mo[0:sz, g * 256:(g + 1) * 256])
                    mv = ln_pool.tile([128, nc.vector.BN_AGGR_DIM], F32, tag="mv",
                                      name="mv")
                    nc.vector.bn_aggr(mv[0:sz, :], st[0:sz, :, :])
                    sd = ln_pool.tile([128, 2], F32, tag="sd", name="sd")
                    nc.scalar.activation(sd[0:sz, 0:1], mv[0:sz, 1:2], AF.Sqrt,
                                         bias=eps_sb[0:sz, :], scale=1.0)
                    rstd = ln_pool.tile([128, 1], F32, tag="rstd", name="rstd")
                    with nc.allow_low_precision(reason="ln2 recip"):
                        nc.vector.reciprocal_approx_fast(rstd[0:sz, :], sd[0:sz, 0:1])
                    nmr = ln_pool.tile([128, 1], F32, tag="nmr", name="nmr")
                    nc.vector.tensor_scalar(nmr[0:sz, :], mv[0:sz, 0:1],
                                            rstd[0:sz, :], -1.0, OP.mult, OP.mult)
                    tln = mo_pool.tile([128, DIM], F32, tag="tln", name="tln")
                    nc.scalar.activation(tln[0:sz, :], mo[0:sz, :], AF.Identity,
                                         bias=nmr[0:sz, :], scale=rstd[0:sz, :])
                    ot = out_pool.tile([128, DIM], F32, tag="ot", name="ot")
                    nc.vector.tensor_tensor(ot[0:sz, :], tln[0:sz, :],
                                            bcast["g2"][0:sz, :], OP.mult)
                    nc.vector.tensor_add(ot[0:sz, :], ot[0:sz, :],
                                         x1_all[0:sz, p, i, :])
                    nc.vector.tensor_add(ot[0:sz, :], ot[0:sz, :],
                                         bcast["be2"][0:sz, :])
                    nc.sync.dma_start(out_d[g0 + off:g0 + off + sz, :], ot[0:sz, :])

            box0 = {}
            for it in x1t_items(0, box0):
                it()
            x1T_cur = box0["x1T"]
            prev_pacs = None
            prev_box = None
            for p in range(NPAIR):
                ev = []
                tl = []
                if p > 0:
                    pbox = {}
                    ev = evict_items_for(p - 1, prev_pacs, pbox)
                    tl = tail_items_for(p - 1, pbox)
                nbox = {}
                nxt_items = x1t_items(p + 1, nbox) if p + 1 < NPAIR else []
                prev_pacs = mlp(p, x1T_cur, ev, tl + nxt_items)
                if p + 1 < NPAIR:
                    x1T_cur = nbox["x1T"]
            # final tail
            fbox = {}
            for it in evict_items_for(NPAIR - 1, prev_pacs, fbox):
                it()
            for it in tail_items_for(NPAIR - 1, fbox):
                it()

    nc.compile()
    return nc


def kernel(x, Wq, bq, Wk, bk, Wv, bv, Wo, bo, W1, b1, W2, b2, g1, be1, g2, be2):
    global _cached
    if _cached is None:
        _cached = _build()
    nc = _cached

    bf = ml_dtypes.bfloat16

    def shuf(w):
        # [C*128, out] -> [128, C*out] partition-major
        w = np.asarray(w, np.float32).astype(bf)
        cdim = w.shape[0] // 128
        return np.ascontiguousarray(
            w.reshape(cdim, 128, w.shape[1]).transpose(1, 0, 2).reshape(128, -1))

    weights = {name: shuf(arr) for name, arr in
               [("Wq", Wq), ("Wk", Wk), ("Wv", Wv), ("Wo", Wo),
                ("W1", W1), ("W2", W2)]}
    bcb = np.stack([np.asarray(a, np.float32).astype(bf)
                    for a in [bv, bo, b2, g1, be1, g2, be2]])  # [7, DIM]
    weights["bcb"] = np.ascontiguousarray(
        np.broadcast_to(bcb[None], (128, 7, DIM)).reshape(128, 7 * DIM))
    bpp = np.concatenate([
        np.asarray(bq, np.float32).reshape(DC, 128).T,
        np.asarray(bk, np.float32).reshape(DC, 128).T,
        np.asarray(b1, np.float32).reshape(HC, 128).T], axis=1)  # [128, 36]
    weights["bpp"] = np.ascontiguousarray(bpp)
    x = np.asarray(x, np.float32).astype(bf)

    in_maps = []
    for c in range(N_CORES):
        xc = np.ascontiguousarray(x[c * BPC:(c + 1) * BPC].reshape(T, DIM))
        xtc = shuf(xc.T.astype(np.float32))  # [768,T] -> [128, 6*T]
        in_maps.append({"x": xc, "xt": xtc, **weights})

    res = run_bass_kernel_spmd(nc, in_maps, core_ids=list(range(N_CORES)),
                               trace=bool(int(os.environ.get("BASSK_TRACE", "0"))))
    kernel._last_res = res
    out = np.concatenate(
        [res.results[c]["out"].reshape(BPC, S, DIM) for c in range(N_CORES)], axis=0)
    return out.astype(np.float32)


# revision 18
# speedup vs baseline: 1.0326x; 1.0326x over previous
"""Trainium2 Bass kernel for a ViT-Base transformer encoder block.

Input x: [64, 197, 768] fp32 + weights. Data-parallel over batch across 8
NeuronCores (8 batches/core = 1576 tokens/core). All matmul operands are
bf16 (fp32 PSUM accumulation); weights and x are cast to bf16 host-side,
and x is additionally passed pre-transposed (d-major) so no PE transposes
are needed in pass 1.

Per core, two passes over 4 batch-pairs (2 batches = 394 tokens each):

  pass 1: QKV projections, software-pipelined attention (per-batch
          197-col matmuls; odd heads write PSUM partitions 64:128 via
          tile_position; softmax denominators via rowsum matmuls +
          reciprocal_approx_fast + gpsimd partition_broadcast),
          O-projection, LayerNorm1 + residual -> x1 kept in SBUF (bf16).
          Pair p+1's projections are interleaved into pair p's attention
          pipeline to keep the in-order PE queue dense.
  pass 2: MLP with W1/W2 resident in SBUF (bf16), exact GELU fused into
          the PSUM eviction, PE transpose back to token-major,
          LayerNorm2 + residual -> out.
"""
import os
import sys

sys.path.insert(0, "/opt/trn_rl_repo")

import numpy as np
import ml_dtypes
from contextlib import ExitStack

import concourse.bass as bass
import concourse.tile as tile
from concourse import bacc, mybir
from concourse.bass_utils import run_bass_kernel_spmd
from concourse.masks import make_identity

DIM, NH, HD, HID = 768, 12, 64, 3072
S = 197
B = 64
N_CORES = 8
BPC = B // N_CORES            # 8 batches per core
T = BPC * S                   # 1576 tokens per core
NPAIR = BPC // 2              # 4 batch pairs per core
PT = 2 * S                    # 394 tokens per pair
EPS = 1e-6
DC = DIM // 128               # 6 d-chunks
HC = HID // 128               # 24 hidden chunks

F32 = mybir.dt.float32
BF16 = mybir.dt.bfloat16
AF = mybir.ActivationFunctionType
OP = mybir.AluOpType

# token tiles within a pair: (offset, size); tile 2*b + s for batch b
Q_TILES = [(0, 128), (128, 69), (197, 128), (325, 69)]

DEBUG = bool(int(os.environ.get("BASSK_DEBUG", "0")))

_cached = None


def _build():
    nc = bacc.Bacc("TRN2", target_bir_lowering=False, debug=False)

    # host-side pre-shuffled layouts: weights [128, C*out], xt [128, DC*T],
    # broadcast biases pre-tiled [128, 7*DIM], per-partition biases [128, 36]
    x_d = nc.dram_tensor("x", [T, DIM], BF16, kind="ExternalInput").ap()
    xt_d = nc.dram_tensor("xt", [128, DC * T], BF16, kind="ExternalInput").ap()
    w_d = {}
    for name, shape, dt in [("Wq", [128, DC * DIM], BF16),
                            ("Wk", [128, DC * DIM], BF16),
                            ("Wv", [128, DC * DIM], BF16),
                            ("Wo", [128, DC * DIM], BF16),
                            ("W1", [128, DC * HID], BF16),
                            ("W2", [128, HC * DIM], BF16),
                            ("bcb", [128, 7 * DIM], BF16),
                            ("bpp", [128, 36], F32)]:
        w_d[name] = nc.dram_tensor(name, shape, dt, kind="ExternalInput").ap()
    out_d = nc.dram_tensor("out", [T, DIM], F32, kind="ExternalOutput").ap()

    dbg = {}
    if DEBUG:
        for name, shape in [("dq", [DIM, PT]), ("dk", [DIM, PT]),
                            ("dv", [PT, DIM]), ("dctx", [DIM, PT]),
                            ("dx1", [PT, DIM]), ("dh", [HID, PT]),
                            ("dex", [NH, 2, 128, PT])]:
            dbg[name] = nc.dram_tensor(name, shape, BF16, kind="ExternalOutput").ap()
        dbg["drt"] = nc.dram_tensor("drt", [NH, PT], F32, kind="ExternalOutput").ap()
        dbg["dpc"] = nc.dram_tensor("dpc", [DC, 128, PT], F32, kind="ExternalOutput").ap()

    with tile.TileContext(nc) as tc, ExitStack() as octx:
        persist = octx.enter_context(tc.tile_pool(name="persist", bufs=1))

        # ---- constants ----
        ident_f = persist.tile([128, 128], F32)
        make_identity(nc, ident_f[:])
        ident_b = persist.tile([128, 128], BF16)
        nc.vector.tensor_copy(ident_b[:], ident_f[:])
        ones_b = persist.tile([128, 1], BF16)
        nc.vector.memset(ones_b[:], 1.0)
        eps_sb = persist.tile([128, 1], F32)
        nc.vector.memset(eps_sb[:], EPS)

        # per-partition bias layouts, one packed DMA (bq | bk | b1)
        bpp = persist.tile([128, 36], F32)
        nc.sync.dma_start(bpp[:], w_d["bpp"])
        bq_sb = bpp[:, 0:DC]
        bk_sb = bpp[:, DC:2 * DC]
        b1_sb = bpp[:, 2 * DC:2 * DC + HC]

        # broadcast-[128, 768] biases (bf16), one packed DMA
        bcb = persist.tile([128, 7, DIM], BF16)
        nc.sync.dma_start(bcb[:], w_d["bcb"].rearrange("p (a j) -> p a j", j=DIM))
        bcast = {name: bcb[:, i, :]
                 for i, name in enumerate(
                     ["bv", "bo", "b2", "g1", "be1", "g2", "be2"])}

        # attention weights, resident bf16 (DMAs issued after the x loads,
        # on the scalar-engine queue so they overlap the x DMAs on sync)
        Wt = {}
        for name in ["Wq", "Wk", "Wv", "Wo"]:
            wt = persist.tile([128, DC, DIM], BF16, name=f"wt_{name}", tag=f"wt_{name}")
            Wt[name] = wt

        # W1 prefetched during pass 1 (needed at pass-2 start)
        W1t = persist.tile([128, DC, HID], BF16)

        # x1 = attn block output, kept in SBUF across passes (bf16)
        x1_all = persist.tile([128, NPAIR, 4, DIM], BF16)

        xt_view = xt_d.rearrange("p (c t) -> p c t", t=T)

        # =========================== PASS 1 ===========================
        with ExitStack() as ctx:
            xp = ctx.enter_context(tc.tile_pool(name="xp", bufs=3))
            big = ctx.enter_context(tc.tile_pool(name="p1big", bufs=2))
            exp_pool = ctx.enter_context(tc.tile_pool(name="exp", bufs=6))
            rt_pool = ctx.enter_context(tc.tile_pool(name="rt", bufs=2))
            bc_pool = ctx.enter_context(tc.tile_pool(name="bc", bufs=3))
            ao_pool = ctx.enter_context(tc.tile_pool(name="ao", bufs=2))
            ln_pool = ctx.enter_context(tc.tile_pool(name="ln", bufs=3))

            xfp = ctx.enter_context(tc.tile_pool(name="xfp", bufs=1))
            # full d-major x, loaded once (chunked DMAs for fast queue drain)
            xT_full = xfp.tile([128, DC, T], BF16)

            ps_mm = ctx.enter_context(tc.tile_pool(name="psmm", bufs=2, space="PSUM"))
            ps_sp = ctx.enter_context(tc.tile_pool(name="pssp", bufs=4, space="PSUM"))
            ps_cx = ctx.enter_context(tc.tile_pool(name="pscx", bufs=2, space="PSUM"))

            def load_x(p):
                g0 = p * PT
                x_sb = xp.tile([128, 4, DIM], BF16, tag="x", name="x_sb")
                for i, (off, sz) in enumerate(Q_TILES):
                    nc.sync.dma_start(x_sb[0:sz, i, :], x_d[g0 + off:g0 + off + sz, :])
                return x_sb

            # --- projection work items for one pair (list of thunks) ---
            def proj_items(p, xT, dst):
                g0 = p * PT
                qT = big.tile([128, DC, PT], BF16, tag="qT", name="qT")
                kT = big.tile([128, DC, PT], BF16, tag="kT", name="kT")
                v_sb = big.tile([128, 4, NH, HD], BF16, tag="v", name="v_sb")
                dst["qT"], dst["kT"], dst["v_sb"] = qT, kT, v_sb
                items = []

                def qk_item(wname, bsb, dstT, c):
                    def run():
                        pm = ps_mm.tile([128, 512], F32, tag="mm", name="pm")
                        for kc in range(DC):
                            nc.tensor.matmul(pm[:, 0:PT],
                                             Wt[wname][:, kc, c * 128:(c + 1) * 128],
                                             xT[:, kc, g0:g0 + PT],
                                             start=(kc == 0), stop=(kc == DC - 1))
                        with nc.allow_low_precision(reason="qk bf16"):
                            nc.vector.tensor_scalar(dstT[:, c, :], pm[:, 0:PT],
                                                    bsb[:, c:c + 1], None, OP.add)
                    return run

                def v_item(i, s):
                    def run():
                        off, sz = Q_TILES[i]
                        pm = ps_mm.tile([128, 512], F32, tag="mm", name="pm")
                        for kc in range(DC):
                            nc.tensor.matmul(pm[0:sz, 0:384],
                                             xT[:, kc, g0 + off:g0 + off + sz],
                                             Wt["Wv"][:, kc, s * 384:(s + 1) * 384],
                                             start=(kc == 0), stop=(kc == DC - 1))
                        with nc.allow_low_precision(reason="v bf16"):
                            nc.vector.tensor_add(
                                v_sb[0:sz, i, 6 * s:6 * s + 6, :],
                                pm[0:sz, 0:384].rearrange("p (a b) -> p a b", a=6),
                                bcast["bv"][0:sz, s * 384:(s + 1) * 384]
                                    .rearrange("p (a b) -> p a b", a=6))
                    return run

                for c in range(DC):
                    items.append(qk_item("Wq", bq_sb, qT, c))
                    items.append(qk_item("Wk", bk_sb, kT, c))
                for i in range(4):
                    for s in range(2):
                        items.append(v_item(i, s))
                return items

            # --- attention for pair p, interleaving `items` (next pair's
            #     projections) between pipeline steps ---
            def attention(p, cur, items):
                qT, kT, v_sb = cur["qT"], cur["kT"], cur["v_sb"]
                ctxT = big.tile([128, DC, PT], BF16, tag="ctxT", name="ctxT")
                cur["ctxT"] = ctxT
                it = 0
                NSTEP = NH + 2
                exps = {}   # h -> [exp_s0, exp_s1]
                pcs = {}    # hc -> psum tile
                bcs = {}    # h -> bc tile
                for step in range(NSTEP):
                    # stage S: scores + exp for head `step`
                    if step < NH:
                        h = step
                        hc, hp = h // 2, (h % 2) * 64
                        exps[h] = []
                        for s in range(2):
                            ksz = Q_TILES[s][1]
                            psc = ps_sp.tile([128, 512], F32, tag="sp", name="psc")
                            for b in range(2):
                                koff = Q_TILES[2 * b + s][0]
                                cs = slice(b * S, (b + 1) * S)
                                nc.tensor.matmul(
                                    psc[0:ksz, cs],
                                    kT[hp:hp + 64, hc, koff:koff + ksz],
                                    qT[hp:hp + 64, hc, cs],
                                    start=True, stop=True,
                                    skip_group_check=True)
                            et = exp_pool.tile([128, 394], BF16, tag="exp", name="et")
                            with nc.allow_low_precision(reason="softmax exp bf16"):
                                nc.scalar.activation(et[0:ksz, :], psc[0:ksz, 0:PT],
                                                     AF.Exp, bias=0.0, scale=0.125)
                            exps[h].append(et)
                            if DEBUG and p == 0:
                                nc.sync.dma_start(dbg["dex"][h, s, :, :], et[:, :])
                    # stage R: rowsum + recip + broadcast for head step-1
                    if 1 <= step <= NH:
                        h = step - 1
                        hc, hp = h // 2, (h % 2) * 64
                        pq = ps_sp.tile([128, 512], F32, tag="sp", name="pq")
                        for s in range(2):
                            ksz = Q_TILES[s][1]
                            nc.tensor.matmul(
                                pq[0:1, 0:PT],
                                ones_b[0:ksz, 0:1],
                                exps[h][s][0:ksz, :],
                                start=(s == 0), stop=(s == 1),
                                skip_group_check=True)
                        rt = rt_pool.tile([128, 394], F32, tag="rt", name="rt")
                        with nc.allow_low_precision(reason="softmax recip"):
                            nc.vector.reciprocal_approx_fast(
                                rt[0:1, :], pq[0:1, 0:PT])
                        if DEBUG and p == 0:
                            nc.sync.dma_start(dbg["drt"][h, :], rt[0:1, :])
                        bc = bc_pool.tile([128, 394], F32, tag="bcsb", name="bc")
                        nc.gpsimd.partition_broadcast(bc[:, :], rt[0:1, :],
                                                      channels=128)
                        bcs[h] = bc
                    # stage C: ctx for head step-1
                    if 1 <= step <= NH:
                        h = step - 1
                        hc, hp = h // 2, (h % 2) * 64
                        if hp == 0:
                            pcs[hc] = ps_cx.tile([128, 512], F32, tag="cx",
                                                 name=f"cx{hc}")
                        pc = pcs[hc]
                        for b in range(2):
                            cs = slice(b * S, (b + 1) * S)
                            for s in range(2):
                                ksz = Q_TILES[s][1]
                                nc.tensor.matmul(
                                    pc[hp:hp + 64, cs],
                                    v_sb[0:ksz, 2 * b + s, h, :],
                                    exps[h][s][0:ksz, cs],
                                    start=(s == 0), stop=(s == 1),
                                    skip_group_check=True)
                    # interleave next-pair projection work
                    for _ in range(1):
                        if it < len(items):
                            items[it]()
                            it += 1
                    # stage N: normalize head step-2
                    if step >= 2:
                        h = step - 2
                        hc, hp = h // 2, (h % 2) * 64
                        if DEBUG and p == 0 and hp == 64:
                            dt_ = bc_pool.tile([128, 394], F32, tag="bcsb",
                                               name="dt_")
                            nc.scalar.activation(dt_[:, :], pcs[hc][:, 0:PT],
                                                 AF.Copy, bias=0.0, scale=1.0)
                            nc.sync.dma_start(dbg["dpc"][hc, :, :], dt_[:, :])
                        with nc.allow_low_precision(reason="ctx bf16"):
                            nc.vector.tensor_tensor(
                                ctxT[hp:hp + 64, hc, :],
                                pcs[hc][hp:hp + 64, 0:PT],
                                bcs[h][hp:hp + 64, :], OP.mult)
                return items[it:]

            def o_proj_ln1(p, cur, items):
                it = 0
                ctxT, x_sb = cur["ctxT"], cur["x_sb"]
                for i, (off, sz) in enumerate(Q_TILES):
                    ao = ao_pool.tile([128, DIM], F32, tag="ao", name="ao")
                    for s in range(2):
                        pm = ps_mm.tile([128, 512], F32, tag="mm", name="pm")
                        for c in range(DC):
                            nc.tensor.matmul(pm[0:sz, 0:384],
                                             ctxT[:, c, off:off + sz],
                                             Wt["Wo"][:, c, s * 384:(s + 1) * 384],
                                             start=(c == 0), stop=(c == DC - 1))
                        nc.vector.tensor_add(ao[0:sz, s * 384:(s + 1) * 384],
                                             pm[0:sz, 0:384],
                                             bcast["bo"][0:sz, s * 384:(s + 1) * 384])
                        if it < len(items):
                            items[it]()
                            it += 1
                    # LayerNorm 1
                    st = ln_pool.tile([128, 3, nc.vector.BN_STATS_DIM], F32, tag="st",
                                      name="st")
                    for g in range(3):
                        nc.vector.bn_stats(st[0:sz, g, :], ao[0:sz, g * 256:(g + 1) * 256])
                    mv = ln_pool.tile([128, nc.vector.BN_AGGR_DIM], F32, tag="mv",
                                      name="mv")
                    nc.vector.bn_aggr(mv[0:sz, :], st[0:sz, :, :])
                    sd = ln_pool.tile([128, 2], F32, tag="sd", name="sd")
                    nc.scalar.activation(sd[0:sz, 0:1], mv[0:sz, 1:2], AF.Sqrt,
                                         bias=eps_sb[0:sz, :], scale=1.0)
                    rstd = ln_pool.tile([128, 1], F32, tag="rstd", name="rstd")
                    with nc.allow_low_precision(reason="ln1 recip"):
                        nc.vector.reciprocal_approx_fast(rstd[0:sz, :], sd[0:sz, 0:1])
                    nmr = ln_pool.tile([128, 1], F32, tag="nmr", name="nmr")
                    nc.vector.tensor_scalar(nmr[0:sz, :], mv[0:sz, 0:1],
                                            rstd[0:sz, :], -1.0, OP.mult, OP.mult)
                    nc.scalar.activation(ao[0:sz, :], ao[0:sz, :], AF.Identity,
                                         bias=nmr[0:sz, :], scale=rstd[0:sz, :])
                    nc.vector.tensor_tensor(ao[0:sz, :], ao[0:sz, :],
                                            bcast["g1"][0:sz, :], OP.mult)
                    nc.vector.tensor_add(ao[0:sz, :], ao[0:sz, :], x_sb[0:sz, i, :])
                    with nc.allow_low_precision(reason="x1 bf16"):
                        nc.vector.tensor_add(x1_all[0:sz, p, i, :], ao[0:sz, :],
                                             bcast["be1"][0:sz, :])
                while it < len(items):
                    items[it]()
                    it += 1

            # ---- pass-1 driver: pipelined over pairs ----
            for c_ in range(0, DC, 2):
                nc.sync.dma_start(xT_full[:, c_, :], xt_view[:, c_, :])
                nc.scalar.dma_start(xT_full[:, c_ + 1, :], xt_view[:, c_ + 1, :])
            for name in ["Wq", "Wk", "Wv", "Wo"]:
                nc.scalar.dma_start(
                    Wt[name][:], w_d[name].rearrange("p (c j) -> p c j", j=DIM))
            nc.gpsimd.dma_start(
                W1t[:], w_d["W1"].rearrange("p (c j) -> p c j", j=HID))
            cur = {}
            x_sb0 = load_x(0)
            nxt_x = load_x(1)
            cur["x_sb"] = x_sb0
            for item in proj_items(0, xT_full, cur):
                item()
            for p in range(NPAIR):
                if p + 1 < NPAIR:
                    nxt = {"x_sb": nxt_x}
                    items = proj_items(p + 1, xT_full, nxt)
                else:
                    nxt = None
                    items = []
                left = attention(p, cur, items)
                if p + 2 < NPAIR:
                    nxt_x = load_x(p + 2)
                o_proj_ln1(p, cur, left)

                if DEBUG and p == 0:
                    for c in range(DC):
                        nc.sync.dma_start(dbg["dq"][c * 128:(c + 1) * 128, :],
                                          cur["qT"][:, c, :])
                        nc.sync.dma_start(dbg["dk"][c * 128:(c + 1) * 128, :],
                                          cur["kT"][:, c, :])
                        nc.sync.dma_start(dbg["dctx"][c * 128:(c + 1) * 128, :],
                                          cur["ctxT"][:, c, :])
                    for i, (off, sz) in enumerate(Q_TILES):
                        nc.sync.dma_start(dbg["dv"][off:off + sz, :],
                                          cur["v_sb"][0:sz, i, :, :])
                        nc.sync.dma_start(dbg["dx1"][off:off + sz, :],
                                          x1_all[0:sz, 0, i, :])
                cur = nxt

        # =========================== PASS 2 ===========================
        with ExitStack() as ctx:
            wpool = ctx.enter_context(tc.tile_pool(name="w2p", bufs=1))
            W2t = wpool.tile([128, HC, DIM], BF16)
            nc.gpsimd.dma_start(W2t[:], w_d["W2"].rearrange("p (c j) -> p c j", j=DIM))

            xtp = ctx.enter_context(tc.tile_pool(name="xtp", bufs=2))
            htp = ctx.enter_context(tc.tile_pool(name="htp", bufs=1))
            mo_pool = ctx.enter_context(tc.tile_pool(name="mo", bufs=2))
            moT_pool = ctx.enter_context(tc.tile_pool(name="moT", bufs=2))
            ln_pool = ctx.enter_context(tc.tile_pool(name="ln2", bufs=3))
            out_pool = ctx.enter_context(tc.tile_pool(name="outp", bufs=2))

            ps_wk = ctx.enter_context(tc.tile_pool(name="pswk", bufs=2, space="PSUM"))
            ps_ac = ctx.enter_context(tc.tile_pool(name="psac", bufs=6, space="PSUM"))

            def x1t_items(p, box):
                x1T = xtp.tile([128, DC, PT], BF16, tag="x1T", name="x1T")
                box["x1T"] = x1T

                def one(i):
                    def run():
                        off, sz = Q_TILES[i]
                        for c in range(DC):
                            pt = ps_wk.tile([128, 512], F32, tag="wk", name="pt")
                            ptb = pt[:, 0:64].bitcast(BF16)
                            nc.tensor.transpose(
                                ptb[:, 0:sz],
                                x1_all[0:sz, p, i, c * 128:(c + 1) * 128],
                                ident_b[0:sz, 0:sz])
                            nc.vector.tensor_copy(x1T[:, c, off:off + sz],
                                                  ptb[:, 0:sz])
                    return run
                return [one(i) for i in range(4)]

            LAG = 6

            def mlp(p, x1T, evict_items, tail_its):
                hT = htp.tile([128, HC, PT], BF16, tag="hT", name="hT")
                pacs = [ps_ac.tile([128, 512], F32, tag="ac", name=f"pac{c}")
                        for c in range(DC)]
                ti = 0
                for hcx in range(HC + LAG):
                    if hcx < HC:
                        pm = ps_wk.tile([128, 512], F32, tag="wk", name="pm")
                        for kc in range(DC):
                            nc.tensor.matmul(pm[:, 0:PT],
                                             W1t[:, kc, hcx * 128:(hcx + 1) * 128],
                                             x1T[:, kc, :],
                                             start=(kc == 0), stop=(kc == DC - 1))
                        with nc.allow_low_precision(reason="h bf16"):
                            nc.scalar.activation(hT[:, hcx, :], pm[:, 0:PT], AF.Gelu,
                                                 bias=b1_sb[:, hcx:hcx + 1], scale=1.0)
                        if hcx < len(evict_items):
                            evict_items[hcx]()
                    h2 = hcx - LAG
                    if h2 >= 0:
                        for c in range(DC):
                            nc.tensor.matmul(pacs[c][:, 0:PT],
                                             W2t[:, h2, c * 128:(c + 1) * 128],
                                             hT[:, h2, :],
                                             start=(h2 == 0), stop=(h2 == HC - 1))
                    if hcx >= LAG and hcx % 2 == 0 and ti < len(tail_its):
                        tail_its[ti]()
                        ti += 1
                while ti < len(tail_its):
                    tail_its[ti]()
                    ti += 1
                if DEBUG and p == 0:
                    for hcx in range(HC):
                        nc.sync.dma_start(dbg["dh"][hcx * 128:(hcx + 1) * 128, :],
                                          hT[:, hcx, :])
                return pacs

            def evict_items_for(p, pacs, box):
                moT = moT_pool.tile([128, DC, PT], BF16, tag="moT", name="moT")
                box["moT"] = moT

                def one(c):
                    def run():
                        with nc.allow_low_precision(reason="moT bf16"):
                            if c % 2 == 0:
                                nc.scalar.activation(moT[:, c, :], pacs[c][:, 0:PT],
                                                     AF.Copy, bias=0.0, scale=1.0)
                            else:
                                nc.vector.tensor_copy(moT[:, c, :], pacs[c][:, 0:PT])
                    return run
                return [one(c) for c in range(DC)]

            def tail_items_for(p, box):
                g0 = p * PT
                moT = box["moT"]

                def one(i):
                    def run():
                        off, sz = Q_TILES[i]
                        _ln2_tile(p, g0, moT, i, off, sz)
                    return run
                return [one(i) for i in range(4)]

            def _ln2_tile(p, g0, moT, i, off, sz):
                if True:
                    mo = mo_pool.tile([128, DIM], F32, tag="mo", name="mo")
                    for c in range(DC):
                        pt = ps_wk.tile([128, 512], F32, tag="wk", name="pt")
                        ptb = pt[:, 0:64].bitcast(BF16)
                        nc.tensor.transpose(ptb[0:sz, 0:128],
                                            moT[:, c, off:off + sz], ident_b[:, :])
                        nc.vector.tensor_copy(mo[0:sz, c * 128:(c + 1) * 128],
                                              ptb[0:sz, 0:128])
                    nc.vector.tensor_add(mo[0:sz, :], mo[0:sz, :], bcast["b2"][0:sz, :])
                    # LayerNorm 2 + residual
                    st = ln_pool.tile([128, 3, nc.vector.BN_STATS_DIM], F32, tag="st",
                                      name="st")
                    for g in range(3):
                        nc.vector.bn_stats(st[0:sz, g, :], mo[0:sz, g * 256:(g + 1) * 256])
                    mv = ln_pool.tile([128, nc.vector.BN_AGGR_DIM], F32, tag="mv",
                                      name="mv")
                    nc.vector.bn_aggr(mv[0:sz, :], st[0:sz, :, :])
                    sd = ln_pool.tile([128, 2], F32, tag="sd", name="sd")
                    nc.scalar.activation(sd[0:sz, 0:1], mv[0:sz, 1:2], AF.Sqrt,
                                         bias=eps_sb[0:sz, :], scale=1.0)
                    rstd = ln_pool.tile([128, 1], F32, tag="rstd", name="rstd")
                    with nc.allow_low_precision(reason="ln2 recip"):
                        nc.vector.reciprocal_approx_fast(rstd[0:sz, :], sd[0:sz, 0:1])
                    nmr = ln_pool.tile([128, 1], F32, tag="nmr", name="nmr")
                    nc.vector.tensor_scalar(nmr[0:sz, :], mv[0:sz, 0:1],
                                            rstd[0:sz, :], -1.0, OP.mult, OP.mult)
                    tln = mo_pool.tile([128, DIM], F32, tag="tln", name="tln")
                    nc.scalar.activation(tln[0:sz, :], mo[0:sz, :], AF.Identity,
                                         bias=nmr[0:sz, :], scale=rstd[0:sz, :])
                    ot = out_pool.tile([128, DIM], F32, tag="ot", name="ot")
                    nc.vector.tensor_tensor(ot[0:sz, :], tln[0:sz, :],
                                            bcast["g2"][0:sz, :], OP.mult)
                    nc.vector.tensor_add(ot[0:sz, :], ot[0:sz, :],
                                         x1_all[0:sz, p, i, :])
                    nc.vector.tensor_add(ot[0:sz, :], ot[0:sz, :],
                                         bcast["be2"][0:sz, :])
                    nc.sync.dma_start(out_d[g0 + off:g0 + off + sz, :], ot[0:sz, :])

            box0 = {}
            for it in x1t_items(0, box0):
                it()
            x1T_cur = box0["x1T"]
            prev_pacs = None
            prev_box = None
            for p in range(NPAIR):
                ev = []
                tl = []
                if p > 0:
                    pbox = {}
                    ev = evict_items_for(p - 1, prev_pacs, pbox)
                    tl = tail_items_for(p - 1, pbox)
                nbox = {}
                nxt_items = x1t_items(p + 1, nbox) if p + 1 < NPAIR else []
                prev_pacs = mlp(p, x1T_cur, ev, tl + nxt_items)
                if p + 1 < NPAIR:
                    x1T_cur = nbox["x1T"]
            # final tail
            fbox = {}
            for it in evict_items_for(NPAIR - 1, prev_pacs, fbox):
                it()
            for it in tail_items_for(NPAIR - 1, fbox):
                it()

    nc.compile()
    return nc


def kernel(x, Wq, bq, Wk, bk, Wv, bv, Wo, bo, W1, b1, W2, b2, g1, be1, g2, be2):
    global _cached
    if _cached is None:
        _cached = _build()
    nc = _cached

    bf = ml_dtypes.bfloat16

    def shuf(w):
        # [C*128, out] -> [128, C*out] partition-major
        w = np.asarray(w, np.float32).astype(bf)
        cdim = w.shape[0] // 128
        return np.ascontiguousarray(
            w.reshape(cdim, 128, w.shape[1]).transpose(1, 0, 2).reshape(128, -1))

    weights = {name: shuf(arr) for name, arr in
               [("Wq", Wq), ("Wk", Wk), ("Wv", Wv), ("Wo", Wo),
                ("W1", W1), ("W2", W2)]}
    bcb = np.stack([np.asarray(a, np.float32).astype(bf)
                    for a in [bv, bo, b2, g1, be1, g2, be2]])  # [7, DIM]
    weights["bcb"] = np.ascontiguousarray(
        np.broadcast_to(bcb[None], (128, 7, DIM)).reshape(128, 7 * DIM))
    bpp = np.concatenate([
        np.asarray(bq, np.float32).reshape(DC, 128).T,
        np.asarray(bk, np.float32).reshape(DC, 128).T,
        np.asarray(b1, np.float32).reshape(HC, 128).T], axis=1)  # [128, 36]
    weights["bpp"] = np.ascontiguousarray(bpp)
    x = np.asarray(x, np.float32).astype(bf)

    in_maps = []
    for c in range(N_CORES):
        xc = np.ascontiguousarray(x[c * BPC:(c + 1) * BPC].reshape(T, DIM))
        xtc = shuf(xc.T.astype(np.float32))  # [768,T] -> [128, 6*T]
        in_maps.append({"x": xc, "xt": xtc, **weights})

    res = run_bass_kernel_spmd(nc, in_maps, core_ids=list(range(N_CORES)),
                               trace=bool(int(os.environ.get("BASSK_TRACE", "0"))))
    kernel._last_res = res
    out = np.concatenate(
        [res.results[c]["out"].reshape(BPC, S, DIM) for c in range(N_CORES)], axis=0)
    return out.astype(np.float32)
